# revision 8
# baseline (speedup 1.0000x reference)
"""Trainium2 Bass kernel for a CQT (constant-Q transform) nn.Module.

Reference computation (per batch sample b, channel c):
    out[b, c, k, f, 0] = sum_t x[b, c, f*HOP + t] * w_re[k, t]
    out[b, c, k, f, 1] = sum_t x[b, c, f*HOP + t] * w_im[k, t]
where w_re/w_im are Hann-windowed complex exponentials with per-bin ragged
lengths (longest 11340 samples), HOP=512, 84 bins, 409 frames.

Strategy: data-parallel over the batch (1 sample per NeuronCore, 8 cores).
Per core the correlation is a banded matmul: the contraction axis t is split
into 89 chunks of 128; chunk c needs x samples x[(f + c//4)*512 + (c%4)*128 + r].
The signal is laid out once in SBUF as Xt[r, ch, rc, m] = x[ch, m*512+rc*128+r]
so every chunk's moving operand is just a 409-column slice of a resident tile.

Weight rows are interleaved (re_k, im_k) pairs sorted by descending window
length, so the rows active in a chunk are always a prefix. Rows 0..127
(bins 0..63) form accumulation group G1 (89 chunks); rows 128..167
(bins 64..83, windows <= 281 samples) form group G2 (3 chunks). The weight
matrix is stored column-compacted (only active rows per chunk), cutting the
weight DMA from 7.9 MB to 1.6 MB without changing the matmul cost.
"""

import math
from contextlib import ExitStack

import numpy as np

import concourse.bass as bass
import concourse.mybir as mybir
import concourse.tile as tile
from concourse import bacc
from concourse.bass_utils import run_bass_kernel_spmd

# ---- problem constants (hardcoded CQT spec) ----
SR = 22050
N_BINS = 84
BPO = 12
FMIN = 32.7
HOP = 512
B, C, T = 8, 2, 220500
N_CORES = 8

LMAX = 11340           # longest window
F = 409                # frames: 1 + (T - LMAX)//HOP
NCHUNK = 89            # ceil(LMAX/128) contraction chunks
MBLK = 432             # 512-sample blocks of x: (F-1)+(NCHUNK-1)//4+1 = 431, +1 pad for FP=410
FP = 410               # fp32r needs an even moving free dim; frame 409 is junk
NROWS = 2 * N_BINS     # interleaved (re, im) weight rows
G1ROWS = 128           # group 1 = rows 0..127  (bins 0..63)
G2ROWS = NROWS - G1ROWS  # 40 rows (bins 64..83)
WBLK = 12              # chunks per weight DMA block

MM_DT = mybir.dt.float32r  # tensor-engine matmul dtype (full-rate fp32)

_PREP = None
_NC = None
LAST_RESULTS = None


def _params():
    """Host-side constants: compacted weight arrays + chunk geometry."""
    global _PREP
    if _PREP is not None:
        return _PREP

    Q = 1.0 / (2.0 ** (1.0 / BPO) - 1.0)
    freqs = FMIN * 2.0 ** (np.arange(N_BINS, dtype=np.float64) / BPO)
    lengths = np.round(Q * SR / freqs).astype(np.int64)
    assert int(lengths.max()) == LMAX

    t = np.arange(LMAX, dtype=np.float64)
    L = lengths.astype(np.float64)[:, None]
    mask = (t[None, :] < L).astype(np.float64)
    win = 0.5 * (1.0 - np.cos(2.0 * math.pi * t[None, :] / L)) * mask
    phase = (2.0 * math.pi / SR) * freqs[:, None] * t[None, :]
    w_re = (win * np.cos(phase)).astype(np.float32)
    w_im = (-win * np.sin(phase)).astype(np.float32)

    # rows 2k / 2k+1 = re_k / im_k; zero-pad time to NCHUNK*128
    W = np.zeros((NROWS, NCHUNK * 128), dtype=np.float32)
    W[0::2, :LMAX] = w_re
    W[1::2, :LMAX] = w_im
    WT = np.ascontiguousarray(W.T)  # (NCHUNK*128, NROWS)

    n_act = np.array([(lengths > 128 * c).sum() for c in range(NCHUNK)])
    assert n_act[0] == N_BINS and n_act[-1] >= 1
    mG1 = np.minimum(G1ROWS, 2 * n_act).astype(np.int64)
    G2C = math.ceil(int(lengths[G1ROWS // 2]) / 128)  # chunks needed by bin 64
    mG2 = (2 * n_act[:G2C] - G1ROWS).astype(np.int64)
    assert mG2[0] == G2ROWS and (mG2 > 0).all()

    base = np.zeros(NCHUNK + 1, dtype=np.int64)
    base[1:] = np.cumsum(mG1)
    SG1 = int(base[-1])
    g2base = np.zeros(G2C + 1, dtype=np.int64)
    g2base[1:] = np.cumsum(mG2)
    SG2 = int(g2base[-1])

    wg1 = np.zeros((128, SG1), dtype=np.float32)
    for c in range(NCHUNK):
        wg1[:, base[c]:base[c + 1]] = WT[128 * c:128 * (c + 1), :mG1[c]]
    wg2 = np.zeros((128, SG2), dtype=np.float32)
    for c in range(G2C):
        wg2[:, g2base[c]:g2base[c + 1]] = WT[128 * c:128 * (c + 1),
                                             G1ROWS:G1ROWS + mG2[c]]

    _PREP = dict(mG1=mG1, mG2=mG2, G2C=G2C, base=base, g2base=g2base,
                 SG1=SG1, SG2=SG2, wg1=wg1, wg2=wg2)
    return _PREP


def _build_nc():
    p = _params()
    mG1, mG2, G2C = p["mG1"], p["mG2"], p["G2C"]
    base, g2base, SG1, SG2 = p["base"], p["g2base"], p["SG1"], p["SG2"]

    nc = bacc.Bacc(None, target_bir_lowering=False)
    xt_d = nc.dram_tensor("xt", (C, 4, 128, MBLK), MM_DT, kind="ExternalInput")
    wg1_d = nc.dram_tensor("wg1", (128, SG1), MM_DT, kind="ExternalInput")
    wg2_d = nc.dram_tensor("wg2", (128, SG2), MM_DT, kind="ExternalInput")
    out_d = nc.dram_tensor("out", (C, NROWS, F), mybir.dt.float32,
                           kind="ExternalOutput")

    with ExitStack() as ctx:
        tc = ctx.enter_context(tile.TileContext(nc))
        xp = ctx.enter_context(tc.tile_pool(name="xp", bufs=1))
        wp = ctx.enter_context(tc.tile_pool(name="wp", bufs=1))
        op = ctx.enter_context(tc.tile_pool(name="op", bufs=1))
        pp = ctx.enter_context(tc.tile_pool(name="pp", bufs=1, space="PSUM"))

        # signal tiles: one per (channel, 128-offset within a 512 block)
        xt_sb = {}
        for ch in range(C):
            for rc in range(4):
                xtile = xp.tile([128, MBLK], MM_DT, name=f"x_{ch}_{rc}",
                                tag=f"x_{ch}_{rc}")
                nc.sync.dma_start(xtile[:], xt_d[ch, rc])
                xt_sb[ch, rc] = xtile

        # weight tiles: blocked along chunks for DMA/compute overlap
        wblks = []  # (first_chunk, tile)
        for b0 in range(0, NCHUNK, WBLK):
            b1 = min(b0 + WBLK, NCHUNK)
            cols = int(base[b1] - base[b0])
            wtile = wp.tile([128, cols], MM_DT, name=f"w_{b0}", tag=f"w_{b0}")
            nc.sync.dma_start(wtile[:], wg1_d[:, int(base[b0]):int(base[b1])])
            wblks.append((b0, wtile))
        wg2_sb = wp.tile([128, SG2], MM_DT, name="wg2_sb", tag="wg2_sb")
        nc.sync.dma_start(wg2_sb[:], wg2_d[:])

        for ch in range(C):
            ps1 = pp.tile([128, FP], mybir.dt.float32, name=f"ps1_{ch}",
                          tag=f"ps1_{ch}")
            ps2 = pp.tile([128, FP], mybir.dt.float32, name=f"ps2_{ch}",
                          tag=f"ps2_{ch}")
            for c in range(NCHUNK):
                j, rc = divmod(c, 4)
                b0, wtile = wblks[c // WBLK]
                off = int(base[c] - base[b0])
                m = int(mG1[c])
                # ragged prefix accumulation: rows [mG1[c+1], mG1[c]) see
                # their last write before the group's nominal stop, which the
                # sim group checker can't express — data correctness comes
                # from the pending-zero mechanism (start=True on chunk 0
                # zeroes all 128 rows of the bank region).
                nc.tensor.matmul(
                    ps1[0:m, :], wtile[:, off:off + m],
                    xt_sb[ch, rc][:, j:j + FP],
                    start=(c == 0), stop=(c == NCHUNK - 1),
                    skip_group_check=True)
            for c in range(G2C):
                j, rc = divmod(c, 4)
                m = int(mG2[c])
                nc.tensor.matmul(
                    ps2[0:m, :], wg2_sb[:, int(g2base[c]):int(g2base[c]) + m],
                    xt_sb[ch, rc][:, j:j + FP],
                    start=(c == 0), stop=(c == G2C - 1),
                    skip_group_check=True)
            o1 = op.tile([128, F], mybir.dt.float32, name=f"o1_{ch}",
                         tag=f"o1_{ch}")
            o2 = op.tile([G2ROWS, F], mybir.dt.float32, name=f"o2_{ch}",
                         tag=f"o2_{ch}")
            nc.vector.tensor_copy(o1[:], ps1[:, 0:F])
            nc.vector.tensor_copy(o2[:], ps2[0:G2ROWS, 0:F])
            nc.gpsimd.dma_start(out_d[ch, 0:G1ROWS, :], o1[:])
            nc.gpsimd.dma_start(out_d[ch, G1ROWS:NROWS, :], o2[:])
    nc.finalize()
    return nc


def get_nc():
    global _NC
    if _NC is None:
        _NC = _build_nc()
    return _NC


def _pack_x(xb):
    """(C, T) -> (C, 4, 128, MBLK) with xt[ch, rc, r, m] = x[ch, m*512+rc*128+r]."""
    xpad = np.zeros((C, MBLK * 512), dtype=np.float32)
    xpad[:, :T] = xb
    return np.ascontiguousarray(
        xpad.reshape(C, MBLK, 4, 128).transpose(0, 2, 3, 1))


def kernel(x):
    global LAST_RESULTS
    x = np.asarray(x, dtype=np.float32)
    assert x.shape == (B, C, T)
    p = _params()
    in_maps = [{"xt": _pack_x(x[b]), "wg1": p["wg1"], "wg2": p["wg2"]}
               for b in range(B)]
    nc = get_nc()
    res = run_bass_kernel_spmd(nc, in_maps, core_ids=list(range(N_CORES)))
    LAST_RESULTS = res
    out = np.empty((B, C, N_BINS, F, 2), dtype=np.float32)
    for b in range(B):
        raw = np.asarray(res.results[b]["out"])  # (C, NROWS, F)
        out[b] = raw.reshape(C, N_BINS, 2, F).transpose(0, 1, 3, 2)
    return out
